# revision 21
# baseline (speedup 1.0000x reference)
"""Trainium2 Bass kernel for nn_DivTree (moe_routing).

Computation (per reference):
    x1 = relu(x0 @ W_shared + b_shared)         # [B, A, H]
    h  = relu(einsum('bah,ahk', x1, W1[route]) + b1[route])
    y  = einsum('bah,ahk', h, W2[route]) + b2[route]   # [B, A, NA]

Strategy: data-parallel over the batch dim across 8 NeuronCores
(B=4096 -> 512 rows/core), weights replicated. On each core the three
GEMMs run in feature-major layout ([feature, batch] activations) so the
contraction dim always sits on SBUF partitions and weights load in their
natural [in, out] layout as the stationary matmul operand. route has few
distinct experts (8 at tree level 3); agents are grouped by expert so each
expert's W1/W2 is DMA'd once and reused for all of its agents.

v2: all matmul operands in bf16 (fp32 PSUM accumulate, fp32 biases;
final rel err ~4e-3, tolerance 2e-2). bf16 enables the PE's fast weight
load so LDWEIGHTS overlaps the previous matmul's stream, and halves HBM
traffic. The output layer (M=32 per agent) is packed 4 agents per matmul
wave via tile_position col-tiling: 4 concurrent [128k,32]x[128k,512]
matmuls into one PSUM bank at partition offsets 0/32/64/96.
"""

import numpy as np

P = 128
N_CORES = 8

_cache: dict = {}


def _build(A, D, H, NA, Bl, chunks):
    import concourse.mybir as mybir
    import concourse.tile as tile
    from concourse import bacc
    from contextlib import ExitStack

    f32 = mybir.dt.float32
    bf16 = mybir.dt.bfloat16
    Relu = mybir.ActivationFunctionType.Relu
    E = len(chunks)  # number of (expert, <=4 agents) output chunks
    KD, KH, MH = D // P, H // P, H // P
    NB = Bl  # matmul free dim (batch); Bl=512 fits one PSUM bank
    assert NB <= 512 and H % P == 0 and D % P == 0 and NA <= 32

    # all DRAM inputs are partition-major so DMA descriptor lines are
    # 2-4KB contiguous per partition (bf16 at 1KB lines loses ~30% DMA
    # throughput to per-descriptor overhead)
    nc = bacc.Bacc()
    x0t = nc.declare_dram_parameter("x0t", [A, P, KD, Bl], bf16, isOutput=False)
    ws = nc.declare_dram_parameter("ws", [P, KD, H], bf16, isOutput=False)
    bs = nc.declare_dram_parameter("bs", [H], f32, isOutput=False)
    w1g = nc.declare_dram_parameter("w1g", [E, P, KH, H], bf16, isOutput=False)
    b1g = nc.declare_dram_parameter("b1g", [E, H], f32, isOutput=False)
    w2g = nc.declare_dram_parameter("w2g", [E, P, KH, NA], bf16, isOutput=False)
    b2q = nc.declare_dram_parameter("b2q", [E, P, 1], f32, isOutput=False)
    yo = nc.declare_dram_parameter("yo", [E, P, Bl], f32, isOutput=True)

    with tile.TileContext(nc) as tc, ExitStack() as ctx:
        const = ctx.enter_context(tc.tile_pool(name="const", bufs=1))
        wpool = ctx.enter_context(tc.tile_pool(name="wexp", bufs=2))
        w2pool = ctx.enter_context(tc.tile_pool(name="w2e", bufs=E))
        xpool = ctx.enter_context(tc.tile_pool(name="x0", bufs=4))
        x1pool = ctx.enter_context(tc.tile_pool(name="x1", bufs=3))
        # h for ALL agents stays resident (A * 4KB/partition) so the whole
        # L1+L2 phase runs as one uninterrupted matmul stream and the
        # col-tiled L3 blocks run back-to-back at the end
        hpool = ctx.enter_context(tc.tile_pool(name="h", bufs=A + 1))
        opool = ctx.enter_context(tc.tile_pool(name="out", bufs=2))
        psum = ctx.enter_context(tc.tile_pool(name="ps", bufs=3, space="PSUM"))
        psum2 = ctx.enter_context(tc.tile_pool(name="ps2", bufs=3, space="PSUM"))
        psum3 = ctx.enter_context(tc.tile_pool(name="ps3", bufs=2, space="PSUM"))

        # PE warm-up: the HAM clock gate holds the array at 1.2GHz until it
        # has been busy ~3.4us. Burn dummy matmuls during the initial DMA
        # wait so the real matmuls start at full clock.
        dummy = const.tile([P, 128], bf16)
        nc.vector.memset(dummy[:], 0.0)
        dps = psum.tile([64, 128], f32, tag="ps")
        for i in range(40):
            nc.tensor.matmul(dps[:], dummy[:, :64], dummy[:, :128],
                             start=True, stop=True)

        # the first agent's input and the shared weights gate the first
        # matmuls: load them before anything else, in k-subtile pairs
        # (2KB descriptor lines) so the PE can start after the first land
        a0 = chunks[0][1][0]
        x0_first = xpool.tile([P, KD, NB], bf16, tag="x0")
        ws_t = const.tile([P, KD, H], bf16)
        for k0 in range(0, KD, 2):
            nc.sync.dma_start(x0_first[:, k0:k0 + 2, :], x0t[a0][:, k0:k0 + 2, :])
            nc.sync.dma_start(ws_t[:, k0:k0 + 2, :], ws[:, k0:k0 + 2, :])
        bs_t = const.tile([P, MH], f32)
        nc.sync.dma_start(bs_t[:], bs.rearrange("(ms p) -> p ms", p=P))

        def emit_l1(a, x0_t):
            x1_t = x1pool.tile([P, MH, NB], bf16, tag="x1", name=f"x1_{a}")
            for ms in range(MH):
                ps1 = psum.tile([P, NB], f32, tag="ps", name=f"ps1_{a}_{ms}")
                for ks in range(KD):
                    nc.tensor.matmul(
                        ps1[:], ws_t[:, ks, ms * P:(ms + 1) * P], x0_t[:, ks, :],
                        start=(ks == 0), stop=(ks == KD - 1),
                    )
                if ms % 2:
                    nc.vector.tensor_scalar(
                        x1_t[:, ms, :], ps1[:], bs_t[:, ms:ms + 1], 0.0,
                        mybir.AluOpType.add, mybir.AluOpType.max)
                else:
                    nc.scalar.activation(x1_t[:, ms, :], ps1[:], Relu,
                                         bias=bs_t[:, ms:ms + 1])
            return x1_t

        def emit_l2(a, x1_t, wt):
            w1_t, b1_t = wt[0], wt[1]
            h_t = hpool.tile([P, MH, NB], bf16, tag="h", name=f"h_{a}")
            for ms in range(MH):
                ps2 = psum2.tile([P, NB], f32, tag="ps2", name=f"ps2_{a}_{ms}")
                for ks in range(KH):
                    nc.tensor.matmul(
                        ps2[:],
                        w1_t[:, ks, ms * P:(ms + 1) * P],
                        x1_t[:, ks, :],
                        start=(ks == 0), stop=(ks == KH - 1),
                    )
                if ms % 2:
                    nc.vector.tensor_scalar(
                        h_t[:, ms, :], ps2[:], b1_t[:, ms:ms + 1], 0.0,
                        mybir.AluOpType.add, mybir.AluOpType.max)
                else:
                    nc.scalar.activation(h_t[:, ms, :], ps2[:], Relu,
                                         bias=b1_t[:, ms:ms + 1])
            return h_t

        def emit_l3(ci, h_ts, w2_t, b2_t, split_evict=False):
            # col-tiled output layer: one [128,512] PSUM bank holds up to 4
            # agents' [32,512] outputs; 4 concurrent matmuls per k-step on
            # col-groups 0/32/64/96 of the PE array
            na = len(h_ts)
            ps3 = psum3.tile([P, NB], f32, tag="ps3", name=f"ps3_{ci}")
            for ks in range(KH):
                for j in range(na):
                    nc.tensor.matmul(
                        ps3[j * 32:j * 32 + NA, :],
                        w2_t[:, ks, :],
                        h_ts[j][:, ks, :],
                        start=(ks == 0), stop=(ks == KH - 1),
                        tile_position=(0, j * 32),
                    )
            o_t = opool.tile([P, NB], f32, tag="o", name=f"o_{ci}")
            if split_evict:
                # final chunk: evict in halves so the first DMA overlaps
                # the second bias-add
                hb = NB // 2
                for q in range(2):
                    nc.vector.tensor_add(
                        o_t[:, q * hb:(q + 1) * hb],
                        ps3[:, q * hb:(q + 1) * hb],
                        b2_t[:, 0:1].to_broadcast((P, hb)),
                    )
                    nc.sync.dma_start(yo[ci][:, q * hb:(q + 1) * hb],
                                      o_t[:, q * hb:(q + 1) * hb])
            else:
                nc.vector.tensor_add(
                    o_t[:], ps3[:],
                    b2_t[:, 0:1].to_broadcast((P, NB)),
                )
                nc.sync.dma_start(yo[ci], o_t[:])

        # flatten: one entry per agent with its chunk context
        stream = []
        for ci, (s, agents) in enumerate(chunks):
            h_list = []
            for i, a in enumerate(agents):
                stream.append((ci, s, a, h_list, i == len(agents) - 1))

        def load_expert(ci, s):
            w1_t = wpool.tile([P, KH, H], bf16, tag="w1", name=f"w1_{ci}")
            for k0 in range(0, KH, 2):
                nc.sync.dma_start(w1_t[:, k0:k0 + 2, :], w1g[s][:, k0:k0 + 2, :])
            b1_t = wpool.tile([P, MH], f32, tag="b1", name=f"b1_{ci}")
            nc.sync.dma_start(b1_t[:], b1g[s].rearrange("(ms p) -> p ms", p=P))
            w2_t = w2pool.tile([P, KH, NA], bf16, tag="w2", name=f"w2_{ci}")
            nc.sync.dma_start(w2_t[:], w2g[s])
            b2_t = w2pool.tile([P, 1], f32, tag="b2", name=f"b2_{ci}")
            nc.sync.dma_start(b2_t[:], b2q[s])
            return (w1_t, b1_t, w2_t, b2_t)

        pending = None  # (a, x1_t, wt) awaiting L2
        l3s = []        # (ci, h_list, w2_t, b2_t): all L3 runs at the end
        last_ci = None
        wt = None
        for ci, s, a, h_list, is_last in stream:
            if ci != last_ci:
                wt = load_expert(ci, s)
                last_ci = ci
                l3s.append((ci, h_list, wt[2], wt[3]))
            if a == a0:
                x0_t = x0_first
            else:
                x0_t = xpool.tile([P, KD, NB], bf16, tag="x0", name=f"x0_{a}")
                # k-subtile pairs: 2KB descriptor lines, and L1's k0
                # matmul only needs the first pair
                for k0 in range(0, KD, 2):
                    nc.sync.dma_start(x0_t[:, k0:k0 + 2, :],
                                      x0t[a][:, k0:k0 + 2, :])
            # one-agent software pipeline: L1(i+1) runs before L2(i),
            # giving the x1 eviction a full L1 group of slack
            x1_t = emit_l1(a, x0_t)
            if pending is not None:
                pa, px1, pwt, phl = pending
                phl.append(emit_l2(pa, px1, pwt))
            pending = (a, x1_t, wt, h_list)

        pa, px1, pwt, phl = pending
        phl.append(emit_l2(pa, px1, pwt))
        for ci, phl, w2_t, b2_t in l3s:
            emit_l3(ci, phl, w2_t, b2_t,
                    split_evict=(ci == l3s[-1][0]))

    nc.compile()
    return nc


def kernel(x0, W_shared, b_shared, W1, b1, W2, b2, route,
           _trace=False, _tmpdir=None):
    import ml_dtypes
    from concourse.bass_utils import run_bass_kernel_spmd

    bf16 = ml_dtypes.bfloat16
    x0 = np.asarray(x0, dtype=np.float32)
    W_shared = np.asarray(W_shared, dtype=np.float32)
    b_shared = np.asarray(b_shared, dtype=np.float32)
    W1 = np.asarray(W1, dtype=np.float32)
    b1 = np.asarray(b1, dtype=np.float32)
    W2 = np.asarray(W2, dtype=np.float32)
    b2 = np.asarray(b2, dtype=np.float32)
    route = np.asarray(route)

    B, A, D = x0.shape
    H = W_shared.shape[1]
    NA = W2.shape[2]
    Bl = B // N_CORES

    experts, inv = np.unique(route, return_inverse=True)
    # chunks of <=4 agents sharing one expert; each chunk -> one output tile
    chunks = []
    for s in range(len(experts)):
        ag = np.where(inv == s)[0].tolist()
        for i in range(0, len(ag), 4):
            chunks.append((s, tuple(ag[i:i + 4])))
    chunks = tuple(chunks)

    key = (B, A, D, H, NA, chunks)
    nc = _cache.get(key)
    if nc is None:
        nc = _build(A, D, H, NA, Bl,
                    tuple((ci, ag) for ci, (s, ag) in enumerate(chunks)))
        _cache[key] = nc

    # host-side shard + transpose to feature-major partition-major layouts
    # (contiguous 2-4KB per-partition DMA lines), gather distinct experts
    KD, KH = D // P, H // P
    sel = [s for s, ag in chunks]
    x0t = np.ascontiguousarray(
        x0.reshape(N_CORES, Bl, A, KD, P).transpose(0, 2, 4, 3, 1)
    ).astype(bf16)  # [NC, A, P, KD, Bl]
    w1g = np.ascontiguousarray(
        W1[sel].reshape(len(sel), KH, P, H).transpose(0, 2, 1, 3)
    ).astype(bf16)  # [E, P, KH, H]
    b1g = np.ascontiguousarray(b1[sel])
    w2g = np.ascontiguousarray(
        W2[sel].reshape(len(sel), KH, P, NA).transpose(0, 2, 1, 3)
    ).astype(bf16)  # [E, P, KH, NA]
    # per-chunk output bias tiled over the 4 col-strips: [E, 128, 1]
    b2q = np.ascontiguousarray(np.tile(b2[sel], (1, P // NA))[:, :, None])
    ws_b = np.ascontiguousarray(
        W_shared.reshape(KD, P, H).transpose(1, 0, 2)).astype(bf16)

    in_maps = [
        dict(x0t=x0t[c], ws=ws_b, bs=b_shared,
             w1g=w1g, b1g=b1g, w2g=w2g, b2q=b2q)
        for c in range(N_CORES)
    ]
    # the axon-proxied runtime occasionally reports a transient
    # "device unrecoverable" right after another process released the
    # cores; a short-delay retry recovers it
    import time
    last_err = None
    for attempt in range(3):
        try:
            res = run_bass_kernel_spmd(nc, in_maps,
                                       core_ids=list(range(N_CORES)),
                                       trace=_trace, tmpdir=_tmpdir)
            break
        except Exception as e:  # noqa: BLE001
            last_err = e
            time.sleep(5.0 * (attempt + 1))
    else:
        raise last_err
    kernel.last_exec_time_ns = res.exec_time_ns
    yo = np.stack([res.results[c]["yo"] for c in range(N_CORES)])  # [NC,E,128,Bl]
    y = np.empty((N_CORES, Bl, A, NA), np.float32)
    for ci, (s, agents) in enumerate(chunks):
        for j, a in enumerate(agents):
            y[:, :, a, :] = yo[:, ci, j * 32:j * 32 + NA, :].transpose(0, 2, 1)
    return y.reshape(B, A, NA)


# revision 23
# speedup vs baseline: 1.0246x; 1.0246x over previous
"""Trainium2 Bass kernel for nn_DivTree (moe_routing).

Computation (per reference):
    x1 = relu(x0 @ W_shared + b_shared)         # [B, A, H]
    h  = relu(einsum('bah,ahk', x1, W1[route]) + b1[route])
    y  = einsum('bah,ahk', h, W2[route]) + b2[route]   # [B, A, NA]

Strategy: data-parallel over the batch dim across 8 NeuronCores
(B=4096 -> 512 rows/core), weights replicated. On each core the three
GEMMs run in feature-major layout ([feature, batch] activations) so the
contraction dim always sits on SBUF partitions and weights load in their
natural [in, out] layout as the stationary matmul operand. route has few
distinct experts (8 at tree level 3); agents are grouped by expert so each
expert's W1/W2 is DMA'd once and reused for all of its agents.

v2: all matmul operands in bf16 (fp32 PSUM accumulate, fp32 biases;
final rel err ~4e-3, tolerance 2e-2). bf16 enables the PE's fast weight
load so LDWEIGHTS overlaps the previous matmul's stream, and halves HBM
traffic. The output layer (M=32 per agent) is packed 4 agents per matmul
wave via tile_position col-tiling: 4 concurrent [128k,32]x[128k,512]
matmuls into one PSUM bank at partition offsets 0/32/64/96.
"""

import numpy as np

P = 128
N_CORES = 8

_cache: dict = {}


def _build(A, D, H, NA, Bl, chunks):
    import concourse.mybir as mybir
    import concourse.tile as tile
    from concourse import bacc
    from contextlib import ExitStack

    f32 = mybir.dt.float32
    bf16 = mybir.dt.bfloat16
    Relu = mybir.ActivationFunctionType.Relu
    E = len(chunks)  # number of (expert, <=4 agents) output chunks
    KD, KH, MH = D // P, H // P, H // P
    NB = Bl  # matmul free dim (batch); Bl=512 fits one PSUM bank
    assert NB <= 512 and H % P == 0 and D % P == 0 and NA <= 32

    # all DRAM inputs are partition-major so DMA descriptor lines are
    # 2-4KB contiguous per partition (bf16 at 1KB lines loses ~30% DMA
    # throughput to per-descriptor overhead)
    nc = bacc.Bacc()
    x0t = nc.declare_dram_parameter("x0t", [A, P, KD, Bl], bf16, isOutput=False)
    ws = nc.declare_dram_parameter("ws", [P, KD, H], bf16, isOutput=False)
    bs = nc.declare_dram_parameter("bs", [H], f32, isOutput=False)
    w1g = nc.declare_dram_parameter("w1g", [E, P, KH, H], bf16, isOutput=False)
    b1g = nc.declare_dram_parameter("b1g", [E, H], f32, isOutput=False)
    w2g = nc.declare_dram_parameter("w2g", [E, P, KH, NA], bf16, isOutput=False)
    b2q = nc.declare_dram_parameter("b2q", [E, P, 1], f32, isOutput=False)
    yo = nc.declare_dram_parameter("yo", [E, P, Bl], f32, isOutput=True)

    with tile.TileContext(nc) as tc, ExitStack() as ctx:
        const = ctx.enter_context(tc.tile_pool(name="const", bufs=1))
        wpool = ctx.enter_context(tc.tile_pool(name="wexp", bufs=2))
        xpool = ctx.enter_context(tc.tile_pool(name="x0", bufs=4))
        x1pool = ctx.enter_context(tc.tile_pool(name="x1", bufs=3))
        hpool = ctx.enter_context(tc.tile_pool(name="h", bufs=8))
        opool = ctx.enter_context(tc.tile_pool(name="out", bufs=2))
        psum = ctx.enter_context(tc.tile_pool(name="ps", bufs=3, space="PSUM"))
        psum2 = ctx.enter_context(tc.tile_pool(name="ps2", bufs=4, space="PSUM"))
        psum3 = ctx.enter_context(tc.tile_pool(name="ps3", bufs=1, space="PSUM"))

        # PE warm-up: the HAM clock gate holds the array at 1.2GHz until it
        # has been busy ~3.4us. Burn dummy matmuls during the initial DMA
        # wait so the real matmuls start at full clock.
        dummy = const.tile([P, 128], bf16)
        nc.vector.memset(dummy[:], 0.0)
        dps = psum.tile([64, 128], f32, tag="ps")
        for i in range(40):
            nc.tensor.matmul(dps[:], dummy[:, :64], dummy[:, :128],
                             start=True, stop=True)

        # the first agent's input and the shared weights gate the first
        # matmuls: load them before anything else, in k-subtile pairs
        # (2KB descriptor lines) so the PE can start after the first land
        a0 = chunks[0][1][0]
        x0_first = xpool.tile([P, KD, NB], bf16, tag="x0")
        ws_t = const.tile([P, KD, H], bf16)
        for k0 in range(0, KD, 2):
            nc.sync.dma_start(x0_first[:, k0:k0 + 2, :], x0t[a0][:, k0:k0 + 2, :])
            nc.sync.dma_start(ws_t[:, k0:k0 + 2, :], ws[:, k0:k0 + 2, :])
        bs_t = const.tile([P, MH], f32)
        nc.sync.dma_start(bs_t[:], bs.rearrange("(ms p) -> p ms", p=P))

        def emit_l1(a, x0_t):
            x1_t = x1pool.tile([P, MH, NB], bf16, tag="x1", name=f"x1_{a}")
            for ms in range(MH):
                ps1 = psum.tile([P, NB], f32, tag="ps", name=f"ps1_{a}_{ms}")
                for ks in range(KD):
                    nc.tensor.matmul(
                        ps1[:], ws_t[:, ks, ms * P:(ms + 1) * P], x0_t[:, ks, :],
                        start=(ks == 0), stop=(ks == KD - 1),
                    )
                if ms % 2:
                    nc.vector.tensor_scalar(
                        x1_t[:, ms, :], ps1[:], bs_t[:, ms:ms + 1], 0.0,
                        mybir.AluOpType.add, mybir.AluOpType.max)
                else:
                    nc.scalar.activation(x1_t[:, ms, :], ps1[:], Relu,
                                         bias=bs_t[:, ms:ms + 1])
            return x1_t

        def emit_l2(a, x1_t, wt):
            w1_t, b1_t = wt[0], wt[1]
            h_t = hpool.tile([P, MH, NB], bf16, tag="h", name=f"h_{a}")
            for ms in range(MH):
                ps2 = psum2.tile([P, NB], f32, tag="ps2", name=f"ps2_{a}_{ms}")
                for ks in range(KH):
                    nc.tensor.matmul(
                        ps2[:],
                        w1_t[:, ks, ms * P:(ms + 1) * P],
                        x1_t[:, ks, :],
                        start=(ks == 0), stop=(ks == KH - 1),
                    )
                if ms % 2:
                    nc.vector.tensor_scalar(
                        h_t[:, ms, :], ps2[:], b1_t[:, ms:ms + 1], 0.0,
                        mybir.AluOpType.add, mybir.AluOpType.max)
                else:
                    nc.scalar.activation(h_t[:, ms, :], ps2[:], Relu,
                                         bias=b1_t[:, ms:ms + 1])
            return h_t

        def emit_l3(ci, h_ts, w2_t, b2_t, split_evict=False):
            # col-tiled output layer: one [128,512] PSUM bank holds up to 4
            # agents' [32,512] outputs; 4 concurrent matmuls per k-step on
            # col-groups 0/32/64/96 of the PE array
            na = len(h_ts)
            ps3 = psum3.tile([P, NB], f32, tag="ps3", name=f"ps3_{ci}")
            for ks in range(KH):
                for j in range(na):
                    nc.tensor.matmul(
                        ps3[j * 32:j * 32 + NA, :],
                        w2_t[:, ks, :],
                        h_ts[j][:, ks, :],
                        start=(ks == 0), stop=(ks == KH - 1),
                        tile_position=(0, j * 32),
                    )
            o_t = opool.tile([P, NB], f32, tag="o", name=f"o_{ci}")
            if split_evict:
                # final chunk: evict in halves so the first DMA overlaps
                # the second bias-add
                hb = NB // 2
                for q in range(2):
                    nc.vector.tensor_add(
                        o_t[:, q * hb:(q + 1) * hb],
                        ps3[:, q * hb:(q + 1) * hb],
                        b2_t[:, 0:1].to_broadcast((P, hb)),
                    )
                    nc.sync.dma_start(yo[ci][:, q * hb:(q + 1) * hb],
                                      o_t[:, q * hb:(q + 1) * hb])
            else:
                nc.vector.tensor_add(
                    o_t[:], ps3[:],
                    b2_t[:, 0:1].to_broadcast((P, NB)),
                )
                nc.sync.dma_start(yo[ci], o_t[:])

        # flatten: one entry per agent with its chunk context
        stream = []
        for ci, (s, agents) in enumerate(chunks):
            h_list = []
            for i, a in enumerate(agents):
                stream.append((ci, s, a, h_list, i == len(agents) - 1))

        def load_expert(ci, s):
            w1_t = wpool.tile([P, KH, H], bf16, tag="w1", name=f"w1_{ci}")
            for k0 in range(0, KH, 2):
                nc.sync.dma_start(w1_t[:, k0:k0 + 2, :], w1g[s][:, k0:k0 + 2, :])
            b1_t = wpool.tile([P, MH], f32, tag="b1", name=f"b1_{ci}")
            nc.sync.dma_start(b1_t[:], b1g[s].rearrange("(ms p) -> p ms", p=P))
            w2_t = wpool.tile([P, KH, NA], bf16, tag="w2", name=f"w2_{ci}")
            nc.sync.dma_start(w2_t[:], w2g[s])
            b2_t = wpool.tile([P, 1], f32, tag="b2", name=f"b2_{ci}")
            nc.sync.dma_start(b2_t[:], b2q[s])
            return (w1_t, b1_t, w2_t, b2_t)

        n_chunks = len(chunks)
        pending = None  # (a, x1_t, wt, h_list, is_last, ci) awaiting L2
        l3q = []        # deferred-L3 queue: (age, ci, h_list, wt)
        last_ci = None
        wt = None
        for ci, s, a, h_list, is_last in stream:
            if ci != last_ci:
                wt = load_expert(ci, s)
                last_ci = ci
            if a == a0:
                x0_t = x0_first
            else:
                x0_t = xpool.tile([P, KD, NB], bf16, tag="x0", name=f"x0_{a}")
                # k-subtile pairs: 2KB descriptor lines, and L1's k0
                # matmul only needs the first pair
                for k0 in range(0, KD, 2):
                    nc.sync.dma_start(x0_t[:, k0:k0 + 2, :],
                                      x0t[a][:, k0:k0 + 2, :])
            # one-agent software pipeline: L1(i+1) runs before L2(i),
            # giving the x1 eviction a full L1 group of slack
            x1_t = emit_l1(a, x0_t)
            # L3 deferred two agents: the last h eviction hides behind a
            # full L1+L2 of tensor work before L3's final k-step needs it
            if l3q and l3q[0][0] >= 1:
                _, fci, fhl, fwt = l3q.pop(0)
                emit_l3(fci, fhl, fwt[2], fwt[3])
            l3q = [(age + 1, *rest) for age, *rest in l3q]
            if pending is not None:
                pa, px1, pwt, phl, plast, pci = pending
                phl.append(emit_l2(pa, px1, pwt))
                if plast:
                    l3q.append((0, pci, phl, pwt))
            pending = (a, x1_t, wt, h_list, is_last, ci)

        pa, px1, pwt, phl, plast, pci = pending
        phl.append(emit_l2(pa, px1, pwt))
        for _, fci, fhl, fwt in l3q:
            emit_l3(fci, fhl, fwt[2], fwt[3],
                    split_evict=(fci == n_chunks - 1))
        if plast:
            emit_l3(pci, phl, pwt[2], pwt[3],
                    split_evict=(pci == n_chunks - 1))

    nc.compile()
    return nc


def kernel(x0, W_shared, b_shared, W1, b1, W2, b2, route,
           _trace=False, _tmpdir=None):
    import ml_dtypes
    from concourse.bass_utils import run_bass_kernel_spmd

    bf16 = ml_dtypes.bfloat16
    x0 = np.asarray(x0, dtype=np.float32)
    W_shared = np.asarray(W_shared, dtype=np.float32)
    b_shared = np.asarray(b_shared, dtype=np.float32)
    W1 = np.asarray(W1, dtype=np.float32)
    b1 = np.asarray(b1, dtype=np.float32)
    W2 = np.asarray(W2, dtype=np.float32)
    b2 = np.asarray(b2, dtype=np.float32)
    route = np.asarray(route)

    B, A, D = x0.shape
    H = W_shared.shape[1]
    NA = W2.shape[2]
    Bl = B // N_CORES

    experts, inv = np.unique(route, return_inverse=True)
    # chunks of <=4 agents sharing one expert; each chunk -> one output tile
    chunks = []
    for s in range(len(experts)):
        ag = np.where(inv == s)[0].tolist()
        for i in range(0, len(ag), 4):
            chunks.append((s, tuple(ag[i:i + 4])))
    chunks = tuple(chunks)

    key = (B, A, D, H, NA, chunks)
    nc = _cache.get(key)
    if nc is None:
        nc = _build(A, D, H, NA, Bl,
                    tuple((ci, ag) for ci, (s, ag) in enumerate(chunks)))
        _cache[key] = nc

    # host-side shard + transpose to feature-major partition-major layouts
    # (contiguous 2-4KB per-partition DMA lines), gather distinct experts
    KD, KH = D // P, H // P
    sel = [s for s, ag in chunks]
    x0t = np.ascontiguousarray(
        x0.reshape(N_CORES, Bl, A, KD, P).transpose(0, 2, 4, 3, 1)
    ).astype(bf16)  # [NC, A, P, KD, Bl]
    w1g = np.ascontiguousarray(
        W1[sel].reshape(len(sel), KH, P, H).transpose(0, 2, 1, 3)
    ).astype(bf16)  # [E, P, KH, H]
    b1g = np.ascontiguousarray(b1[sel])
    w2g = np.ascontiguousarray(
        W2[sel].reshape(len(sel), KH, P, NA).transpose(0, 2, 1, 3)
    ).astype(bf16)  # [E, P, KH, NA]
    # per-chunk output bias tiled over the 4 col-strips: [E, 128, 1]
    b2q = np.ascontiguousarray(np.tile(b2[sel], (1, P // NA))[:, :, None])
    ws_b = np.ascontiguousarray(
        W_shared.reshape(KD, P, H).transpose(1, 0, 2)).astype(bf16)

    in_maps = [
        dict(x0t=x0t[c], ws=ws_b, bs=b_shared,
             w1g=w1g, b1g=b1g, w2g=w2g, b2q=b2q)
        for c in range(N_CORES)
    ]
    # the axon-proxied runtime occasionally reports a transient
    # "device unrecoverable" right after another process released the
    # cores; a short-delay retry recovers it
    import time
    last_err = None
    for attempt in range(3):
        try:
            res = run_bass_kernel_spmd(nc, in_maps,
                                       core_ids=list(range(N_CORES)),
                                       trace=_trace, tmpdir=_tmpdir)
            break
        except Exception as e:  # noqa: BLE001
            last_err = e
            time.sleep(5.0 * (attempt + 1))
    else:
        raise last_err
    kernel.last_exec_time_ns = res.exec_time_ns
    yo = np.stack([res.results[c]["yo"] for c in range(N_CORES)])  # [NC,E,128,Bl]
    y = np.empty((N_CORES, Bl, A, NA), np.float32)
    for ci, (s, agents) in enumerate(chunks):
        for j, a in enumerate(agents):
            y[:, :, a, :] = yo[:, ci, j * 32:j * 32 + NA, :].transpose(0, 2, 1)
    return y.reshape(B, A, NA)
